# revision 1
# baseline (speedup 1.0000x reference)
"""DeepTraceLayer (GNN message passing w/ LSTM aggregator) on 8 Trainium2 cores.

Strategy (data-parallel over nodes, 2560 padded nodes per core):
  - Keep all state transposed: [H=128 partitions, nodes free]. The LSTM
    recurrence then never needs an on-device transpose.
  - Neighbor gather: dma_gather(transpose=True) on a bf16 copy of h pulls
    h[nbr_idx[n,t]] rows (256B each) from HBM directly into the transposed
    [D=128, nodes] layout used as the matmul moving operand.
  - Gates: per 512-node macrotile, 4 gate blocks x 2 accumulating bf16
    matmuls (W_hh.T blk @ hT  +  W_ih.T blk @ nbrT) into PSUM fp32.
  - All-sigmoid trick: g-gate rows of W/b are pre-scaled by 2 on the host, so
    tanh(a) = 2*sigmoid(2a)-1 and ONE batched sigmoid covers all 4 gate
    blocks (4 PSUM banks) per macrotile. The cell state is stored as
    c~ = 2c so tanh(c) = 2*sigmoid(c~)-1 needs no extra scaling pass.
  - Cell/hidden updates: stock DVE bf16 tensor ops (2x/4x modes).
  - Output: relu(W_out @ [h; agg] + b_out) as two accumulating matmuls per
    macrotile + one ACT relu (bias is per-partition in transposed layout);
    host transposes the [128, N] result back.
"""
import numpy as np
import ml_dtypes

import concourse.mybir as mybir
import concourse.tile as tile
from concourse import bacc
from concourse.bass_utils import run_bass_kernel_spmd
from concourse.tile_rust import add_dep_helper

bf16 = ml_dtypes.bfloat16
dt = mybir.dt

N = 20000
D = 128
H = 128
DEG = 32
NCORES = 8
NPAD = 20480            # padded node count (8 * 2560)
PER_CORE = NPAD // NCORES   # 2560
MT = 512                # macrotile (nodes per PSUM group)
NMT = PER_CORE // MT    # 5
IDXCOLS = PER_CORE // 16          # 160 idx columns per gather
AF = mybir.ActivationFunctionType

_BUILD_CACHE = {}


def _wrap_idx(vals):
    """[PER_CORE] int16 -> [128, IDXCOLS] wrapped (i -> [i%16, i//16]) and
    replicated across the eight 16-partition groups."""
    w = vals.reshape(IDXCOLS, 16).T  # [16, IDXCOLS]
    return np.tile(w, (8, 1))


def _build(gate_bias_nonzero: bool, reps: int = 1, variant: int = 2):
    key = (gate_bias_nonzero, reps, variant)
    if key in _BUILD_CACHE:
        return _BUILD_CACHE[key]
    if variant == 2:
        nc = _build_v2(gate_bias_nonzero, reps)
        _BUILD_CACHE[key] = nc
        return nc

    nc = bacc.Bacc(None, target_bir_lowering=False, debug=False)

    hbf = nc.dram_tensor("hbf", [NPAD, D], dt.bfloat16, kind="ExternalInput")
    # per step t: cols [t*IDXCOLS:(t+1)*IDXCOLS]; block DEG = identity idx
    idxw = nc.dram_tensor("idxw", [128, (DEG + 1) * IDXCOLS], dt.int16,
                          kind="ExternalInput")
    whhT = nc.dram_tensor("whhT", [128, 4 * H], dt.bfloat16, kind="ExternalInput")
    wihT = nc.dram_tensor("wihT", [128, 4 * H], dt.bfloat16, kind="ExternalInput")
    woutT = nc.dram_tensor("woutT", [128, D + H], dt.bfloat16, kind="ExternalInput")
    bout = nc.dram_tensor("bout", [128, 1], dt.float32, kind="ExternalInput")
    bgates = nc.dram_tensor("bgates", [128, 4], dt.float32, kind="ExternalInput")
    outT = nc.dram_tensor("outT", [128, PER_CORE], dt.float32, kind="ExternalOutput")

    with tile.TileContext(nc) as tc:
        with (
            tc.tile_pool(name="cpool", bufs=1) as cpool,
            tc.tile_pool(name="state", bufs=1) as state,
            tc.tile_pool(name="gpool", bufs=3) as gpool,
            tc.tile_pool(name="hinp", bufs=1) as hinp,
            tc.tile_pool(name="psum", bufs=2, space="PSUM") as psum,
            tc.tile_pool(name="spool", bufs=2) as spool,
            tc.tile_pool(name="scp", bufs=2) as scp,
            tc.tile_pool(name="tmp", bufs=4) as tmp,
            tc.tile_pool(name="opool", bufs=1) as opool,
        ):
            idx_sb = cpool.tile([128, (DEG + 1) * IDXCOLS], dt.int16)
            nc.sync.dma_start(out=idx_sb[:], in_=idxw[:])
            whh_sb = cpool.tile([128, 4 * H], dt.bfloat16)
            nc.sync.dma_start(out=whh_sb[:], in_=whhT[:])
            wih_sb = cpool.tile([128, 4 * H], dt.bfloat16)
            nc.sync.dma_start(out=wih_sb[:], in_=wihT[:])
            wout_sb = cpool.tile([128, D + H], dt.bfloat16)
            nc.sync.dma_start(out=wout_sb[:], in_=woutT[:])
            bout_sb = cpool.tile([128, 1], dt.float32)
            nc.sync.dma_start(out=bout_sb[:], in_=bout[:])
            bg_sb = cpool.tile([128, 4], dt.float32)
            nc.sync.dma_start(out=bg_sb[:], in_=bgates[:])

            # transposed original-h for the output stage: identity gather
            hTin = hinp.tile([128, 1, PER_CORE], dt.bfloat16)
            nc.gpsimd.dma_gather(
                out_ap=hTin[:], in_ap=hbf[:],
                idxs_ap=idx_sb[:, DEG * IDXCOLS:(DEG + 1) * IDXCOLS],
                num_idxs=PER_CORE, num_idxs_reg=PER_CORE,
                elem_size=D, transpose=True,
            )

            hT = state.tile([128, PER_CORE], dt.bfloat16)
            ct = state.tile([128, PER_CORE], dt.bfloat16)

            for _rep in range(reps):
                nc.vector.memset(hT[:], 0.0)
                nc.vector.memset(ct[:], 0.0)

                for t in range(DEG):
                    gt = gpool.tile([128, 1, PER_CORE], dt.bfloat16, tag="gt")
                    nc.gpsimd.dma_gather(
                        out_ap=gt[:], in_ap=hbf[:],
                        idxs_ap=idx_sb[:, t * IDXCOLS:(t + 1) * IDXCOLS],
                        num_idxs=PER_CORE, num_idxs_reg=PER_CORE,
                        elem_size=D, transpose=True,
                    )
                    S = spool.tile([128, NMT, 4 * MT], dt.bfloat16, tag="S")
                    for m in range(NMT):
                        msl = slice(m * MT, (m + 1) * MT)
                        ps = psum.tile([128, 4, MT], dt.float32, tag="ps")
                        for blk in range(4):
                            bsl = slice(blk * 128, (blk + 1) * 128)
                            nc.tensor.matmul(out=ps[:, blk, :],
                                             lhsT=whh_sb[:, bsl],
                                             rhs=hT[:, msl],
                                             start=True, stop=False)
                            nc.tensor.matmul(out=ps[:, blk, :],
                                             lhsT=wih_sb[:, bsl],
                                             rhs=gt[:, 0, msl],
                                             start=False, stop=True)
                        if gate_bias_nonzero:
                            for blk in range(4):
                                nc.scalar.activation(
                                    S[:, m, blk * MT:(blk + 1) * MT],
                                    ps[:, blk, :], AF.Sigmoid,
                                    bias=bg_sb[:, blk:blk + 1])
                        else:
                            nc.scalar.activation(
                                S[:, m, :],
                                ps[:].rearrange("p a b -> p (a b)"),
                                AF.Sigmoid)
                        Si = S[:, m, 0:MT]
                        Sf = S[:, m, MT:2 * MT]
                        Sg = S[:, m, 2 * MT:3 * MT]
                        So = S[:, m, 3 * MT:4 * MT]  # noqa: F841 (used below)
                        G = tmp.tile([128, MT], dt.bfloat16, tag="G")
                        nc.vector.tensor_scalar(out=G[:], in0=Sg, scalar1=4.0,
                                                scalar2=-2.0,
                                                op0=mybir.AluOpType.mult,
                                                op1=mybir.AluOpType.add)
                        t1 = tmp.tile([128, MT], dt.bfloat16, tag="t1")
                        nc.vector.tensor_tensor(out=t1[:], in0=Si, in1=G[:],
                                                op=mybir.AluOpType.mult)
                        nc.vector.tensor_tensor(out=ct[:, msl], in0=Sf,
                                                in1=ct[:, msl],
                                                op=mybir.AluOpType.mult)
                        nc.vector.tensor_tensor(out=ct[:, msl], in0=ct[:, msl],
                                                in1=t1[:],
                                                op=mybir.AluOpType.add)
                    sc = scp.tile([128, PER_CORE], dt.bfloat16, tag="sc")
                    nc.scalar.activation(sc[:], ct[:], AF.Sigmoid)
                    for m in range(NMT):
                        msl = slice(m * MT, (m + 1) * MT)
                        Tc = tmp.tile([128, MT], dt.bfloat16, tag="Tc")
                        nc.vector.tensor_scalar(out=Tc[:], in0=sc[:, msl],
                                                scalar1=2.0, scalar2=-1.0,
                                                op0=mybir.AluOpType.mult,
                                                op1=mybir.AluOpType.add)
                        nc.vector.tensor_tensor(out=hT[:, msl],
                                                in0=S[:, m, 3 * MT:4 * MT],
                                                in1=Tc[:],
                                                op=mybir.AluOpType.mult)

            # output stage: outT = relu(WhT.T @ hTin + WaT.T @ hT + b_out)
            out_sb = opool.tile([128, PER_CORE], dt.float32)
            for m in range(NMT):
                msl = slice(m * MT, (m + 1) * MT)
                po = psum.tile([128, 4, MT], dt.float32, tag="ps")
                nc.tensor.matmul(out=po[:, 0, :], lhsT=wout_sb[:, 0:D],
                                 rhs=hTin[:, 0, msl], start=True, stop=False)
                nc.tensor.matmul(out=po[:, 0, :], lhsT=wout_sb[:, D:D + H],
                                 rhs=hT[:, msl], start=False, stop=True)
                nc.scalar.activation(out_sb[:, msl], po[:, 0, :], AF.Relu,
                                     bias=bout_sb[:, 0:1])
            nc.sync.dma_start(out=outT[:], in_=out_sb[:])

    nc.compile()
    _BUILD_CACHE[key] = nc
    return nc


def _build_v2(gate_bias_nonzero: bool, reps: int = 1):
    """Barrier-free variant: per-macrotile chains all the way; tanh(c) via the
    native Tanh ACT function per macrotile (no batched sigma(c~) step barrier,
    no Tc fixup). Cell state is plain c (not 2c)."""
    nc = bacc.Bacc(None, target_bir_lowering=False, debug=False)

    hbf = nc.dram_tensor("hbf", [NPAD, D], dt.bfloat16, kind="ExternalInput")
    idxw = nc.dram_tensor("idxw", [128, (DEG + 1) * IDXCOLS], dt.int16,
                          kind="ExternalInput")
    whhT = nc.dram_tensor("whhT", [128, 4 * H], dt.bfloat16, kind="ExternalInput")
    wihT = nc.dram_tensor("wihT", [128, 4 * H], dt.bfloat16, kind="ExternalInput")
    woutT = nc.dram_tensor("woutT", [128, D + H], dt.bfloat16, kind="ExternalInput")
    bout = nc.dram_tensor("bout", [128, 1], dt.float32, kind="ExternalInput")
    bgates = nc.dram_tensor("bgates", [128, 4], dt.float32, kind="ExternalInput")
    outT = nc.dram_tensor("outT", [128, PER_CORE], dt.float32, kind="ExternalOutput")

    with tile.TileContext(nc) as tc:
        with (
            tc.tile_pool(name="cpool", bufs=1) as cpool,
            tc.tile_pool(name="state", bufs=1) as state,
            tc.tile_pool(name="gpool", bufs=8) as gpool,
            tc.tile_pool(name="hinp", bufs=1) as hinp,
            tc.tile_pool(name="psum", bufs=2, space="PSUM") as psum,
            tc.tile_pool(name="spool", bufs=6) as spool,
            tc.tile_pool(name="tmp", bufs=6) as tmp,
            tc.tile_pool(name="opool", bufs=1) as opool,
        ):
            load_insts = []
            idx_sb = cpool.tile([128, (DEG + 1) * IDXCOLS], dt.int16)
            load_insts.append(nc.sync.dma_start(out=idx_sb[:], in_=idxw[:]))
            whh_sb = cpool.tile([128, 4 * H], dt.bfloat16)
            load_insts.append(nc.sync.dma_start(out=whh_sb[:], in_=whhT[:]))
            wih_sb = cpool.tile([128, 4 * H], dt.bfloat16)
            load_insts.append(nc.sync.dma_start(out=wih_sb[:], in_=wihT[:]))
            wout_sb = cpool.tile([128, D + H], dt.bfloat16)
            load_insts.append(nc.sync.dma_start(out=wout_sb[:], in_=woutT[:]))
            bout_sb = cpool.tile([128, 1], dt.float32)
            load_insts.append(nc.sync.dma_start(out=bout_sb[:], in_=bout[:]))
            bg_sb = cpool.tile([128, 4], dt.float32)
            load_insts.append(nc.sync.dma_start(out=bg_sb[:], in_=bgates[:]))

            # xbar (transpose-gather) discipline: transpose-gathers must not
            # overlap plain DMAs (xbar mode conflict hangs the device), and
            # unbounded gather concurrency overflows the SWDGE ring. Chain
            # each gather after all input loads and after the previous
            # gather. Depth 1 is the only stable configuration: depth-2
            # overlap was tried and crashes the device
            # (NRT_EXEC_UNIT_UNRECOVERABLE), as does any num_idxs > 512.
            GATHER_DEPTH = 1
            recent_gathers = []

            def chain_gather(inst):
                if len(recent_gathers) < GATHER_DEPTH:
                    for li in load_insts:
                        add_dep_helper(inst.ins, li.ins, sync=True,
                                       reason="gather after input loads")
                else:
                    add_dep_helper(inst.ins,
                                   recent_gathers[-GATHER_DEPTH].ins,
                                   sync=True,
                                   reason="bounded xbar gather overlap")
                recent_gathers.append(inst)

            hT = state.tile([128, PER_CORE], dt.bfloat16)
            ct = state.tile([128, PER_CORE], dt.bfloat16)

            for _rep in range(reps):
                nc.vector.memset(hT[:], 0.0)
                nc.vector.memset(ct[:], 0.0)

                for t in range(DEG):
                    for m in range(NMT):
                        msl = slice(m * MT, (m + 1) * MT)
                        # one 512-idx gather per macrotile: larger gathers
                        # overflow the SWDGE descriptor ring and hang the HW
                        gt = gpool.tile([128, 1, MT], dt.bfloat16, tag="gt")
                        chain_gather(nc.gpsimd.dma_gather(
                            out_ap=gt[:], in_ap=hbf[:],
                            idxs_ap=idx_sb[:, t * IDXCOLS + m * (MT // 16):
                                           t * IDXCOLS + (m + 1) * (MT // 16)],
                            num_idxs=MT, num_idxs_reg=MT,
                            elem_size=D, transpose=True,
                        ))
                        ps = psum.tile([128, 4, MT], dt.float32, tag="ps")
                        for blk in range(4):
                            bsl = slice(blk * 128, (blk + 1) * 128)
                            nc.tensor.matmul(out=ps[:, blk, :],
                                             lhsT=whh_sb[:, bsl],
                                             rhs=hT[:, msl],
                                             start=True, stop=False)
                            nc.tensor.matmul(out=ps[:, blk, :],
                                             lhsT=wih_sb[:, bsl],
                                             rhs=gt[:, 0, :],
                                             start=False, stop=True)
                        S = spool.tile([128, 4 * MT], dt.bfloat16, tag="S")
                        if gate_bias_nonzero:
                            for blk in range(4):
                                nc.scalar.activation(
                                    S[:, blk * MT:(blk + 1) * MT],
                                    ps[:, blk, :], AF.Sigmoid,
                                    bias=bg_sb[:, blk:blk + 1])
                        else:
                            nc.scalar.activation(
                                S[:], ps[:].rearrange("p a b -> p (a b)"),
                                AF.Sigmoid)
                        Si = S[:, 0:MT]
                        Sf = S[:, MT:2 * MT]
                        Sg = S[:, 2 * MT:3 * MT]
                        So = S[:, 3 * MT:4 * MT]
                        # G = tanh(a_g) = 2*sigmoid(2 a_g) - 1
                        G = tmp.tile([128, MT], dt.bfloat16, tag="G")
                        nc.vector.tensor_scalar(out=G[:], in0=Sg, scalar1=2.0,
                                                scalar2=-1.0,
                                                op0=mybir.AluOpType.mult,
                                                op1=mybir.AluOpType.add)
                        t1 = tmp.tile([128, MT], dt.bfloat16, tag="t1")
                        nc.vector.tensor_tensor(out=t1[:], in0=Si, in1=G[:],
                                                op=mybir.AluOpType.mult)
                        nc.vector.tensor_tensor(out=ct[:, msl], in0=Sf,
                                                in1=ct[:, msl],
                                                op=mybir.AluOpType.mult)
                        nc.vector.tensor_tensor(out=ct[:, msl], in0=ct[:, msl],
                                                in1=t1[:],
                                                op=mybir.AluOpType.add)
                        Tc = tmp.tile([128, MT], dt.bfloat16, tag="Tc")
                        nc.scalar.activation(Tc[:], ct[:, msl], AF.Tanh)
                        nc.vector.tensor_tensor(out=hT[:, msl], in0=So,
                                                in1=Tc[:],
                                                op=mybir.AluOpType.mult)

            # transposed original-h for the output stage; last links in the
            # gather chain so they never overlap another xbar user.
            hTin = hinp.tile([128, NMT, MT], dt.bfloat16)
            for m in range(NMT):
                chain_gather(nc.gpsimd.dma_gather(
                    out_ap=hTin[:, m:m + 1, :], in_ap=hbf[:],
                    idxs_ap=idx_sb[:, DEG * IDXCOLS + m * (MT // 16):
                                   DEG * IDXCOLS + (m + 1) * (MT // 16)],
                    num_idxs=MT, num_idxs_reg=MT,
                    elem_size=D, transpose=True,
                ))

            out_sb = opool.tile([128, PER_CORE], dt.float32)
            for m in range(NMT):
                msl = slice(m * MT, (m + 1) * MT)
                po = psum.tile([128, 4, MT], dt.float32, tag="ps")
                nc.tensor.matmul(out=po[:, 0, :], lhsT=wout_sb[:, 0:D],
                                 rhs=hTin[:, m, :], start=True, stop=False)
                nc.tensor.matmul(out=po[:, 0, :], lhsT=wout_sb[:, D:D + H],
                                 rhs=hT[:, msl], start=False, stop=True)
                nc.scalar.activation(out_sb[:, msl], po[:, 0, :], AF.Relu,
                                     bias=bout_sb[:, 0:1])
            nc.sync.dma_start(out=outT[:], in_=out_sb[:])

    nc.compile()
    return nc


def _build_null():
    """Same I/O signature as the real kernel but a trivial body — used by
    test.py to measure per-call dispatch overhead (HW time = real - null)."""
    key = ("null",)
    if key in _BUILD_CACHE:
        return _BUILD_CACHE[key]
    nc = bacc.Bacc(None, target_bir_lowering=False, debug=False)
    nc.dram_tensor("hbf", [NPAD, D], dt.bfloat16, kind="ExternalInput")
    nc.dram_tensor("idxw", [128, (DEG + 1) * IDXCOLS], dt.int16,
                   kind="ExternalInput")
    nc.dram_tensor("whhT", [128, 4 * H], dt.bfloat16, kind="ExternalInput")
    nc.dram_tensor("wihT", [128, 4 * H], dt.bfloat16, kind="ExternalInput")
    nc.dram_tensor("woutT", [128, D + H], dt.bfloat16, kind="ExternalInput")
    bout = nc.dram_tensor("bout", [128, 1], dt.float32, kind="ExternalInput")
    nc.dram_tensor("bgates", [128, 4], dt.float32, kind="ExternalInput")
    outT = nc.dram_tensor("outT", [128, PER_CORE], dt.float32,
                          kind="ExternalOutput")
    with tile.TileContext(nc) as tc:
        with tc.tile_pool(name="sbuf", bufs=1) as sbuf:
            b_sb = sbuf.tile([128, 1], dt.float32)
            nc.sync.dma_start(out=b_sb[:], in_=bout[:])
            o_sb = sbuf.tile([128, PER_CORE], dt.float32)
            nc.vector.memset(o_sb[:], 0.0)
            nc.sync.dma_start(out=outT[:], in_=o_sb[:])
    nc.compile()
    _BUILD_CACHE[key] = nc
    return nc


def _prep_inputs(h, nbr_idx, W_ih, W_hh, b_ih, b_hh, W_out, b_out):
    h = np.asarray(h, np.float32)
    nbr_idx = np.asarray(nbr_idx)
    W_ih = np.asarray(W_ih, np.float32).copy()
    W_hh = np.asarray(W_hh, np.float32).copy()
    b = (np.asarray(b_ih, np.float32) + np.asarray(b_hh, np.float32)).copy()
    W_out = np.asarray(W_out, np.float32)
    b_out = np.asarray(b_out, np.float32)

    # all-sigmoid trick: scale g-gate rows (PyTorch order i,f,g,o) by 2
    W_ih[2 * H:3 * H] *= 2.0
    W_hh[2 * H:3 * H] *= 2.0
    b[2 * H:3 * H] *= 2.0

    hpad = np.zeros((NPAD, D), np.float32)
    hpad[:N] = h
    hbf = hpad.astype(bf16)

    idx_pad = np.zeros((NPAD, DEG), np.int16)
    idx_pad[:N] = nbr_idx.astype(np.int16)

    whhT = np.ascontiguousarray(W_hh.T).astype(bf16)      # [128, 512]
    wihT = np.ascontiguousarray(W_ih.T).astype(bf16)      # [128, 512]
    # W_out is [H, D+H]; lhsT [k, m]: col-block 0:D holds Wh.T, D:D+H Wa.T
    wout_tile = np.zeros((128, D + H), np.float32)
    wout_tile[:, 0:D] = W_out[:, 0:D].T        # Wh.T: [D, H] = [128,128]
    wout_tile[:, D:D + H] = W_out[:, D:D + H].T
    wout_tile = wout_tile.astype(bf16)
    bout_t = b_out.reshape(H, 1).astype(np.float32)
    bg = np.ascontiguousarray(b.reshape(4, H).T).astype(np.float32)  # [128,4]
    gate_bias_nonzero = bool(np.any(b != 0.0))

    in_maps = []
    for c in range(NCORES):
        base = c * PER_CORE
        cols = []
        for t in range(DEG):
            cols.append(_wrap_idx(idx_pad[base:base + PER_CORE, t]))
        ident = (base + np.arange(PER_CORE)).astype(np.int16)
        cols.append(_wrap_idx(ident))
        idxw = np.concatenate(cols, axis=1)
        in_maps.append({
            "hbf": hbf, "idxw": idxw, "whhT": whhT, "wihT": wihT,
            "woutT": wout_tile, "bout": bout_t, "bgates": bg,
        })
    return in_maps, gate_bias_nonzero


def kernel(h, nbr_idx, W_ih, W_hh, b_ih, b_hh, W_out, b_out, _reps=1):
    in_maps, gate_bias_nonzero = _prep_inputs(
        h, nbr_idx, W_ih, W_hh, b_ih, b_hh, W_out, b_out)
    nc = _build(gate_bias_nonzero, reps=_reps)
    res = run_bass_kernel_spmd(nc, in_maps, core_ids=list(range(NCORES)),
                               trace=False)
    outT = np.concatenate([res.results[c]["outT"] for c in range(NCORES)],
                          axis=1)  # [128, NPAD]
    return np.ascontiguousarray(outT.T[:N]).astype(np.float32)

